# revision 10
# baseline (speedup 1.0000x reference)
"""Dense causal attention block (QKV proj + RoPE + causal attention + out proj)
distributed over 8 TRN2 NeuronCores.

Sharding: tensor-parallel over heads (2 heads/core). Each core computes
q/k/v for its heads (q/k d-major so QK^T needs no transposes, v
token-major), causal attention in logits-transposed [k, q] layout
(softmax denominator via an all-ones stationary matmul, no max
subtraction -- logits are O(1) -- normalization folded into the PSUM
eviction), then one AllToAll per batch converts head-sharding to
token-sharding so the output projection overlaps the other batch's
attention. Host assembles the 16 token chunks.

All matmuls run in bf16 (f32 PSUM accumulate); host pre-casts and
pre-tiles the weights so every DMA is partition-contiguous.
"""
import numpy as np
import ml_dtypes

import concourse.bass as bass
import concourse.mybir as mybir
from concourse import tile, bacc
from concourse.bass_utils import run_bass_kernel_spmd

BF16 = ml_dtypes.bfloat16
B, S, D = 2, 2048, 2048
H, HD = 16, 128
NCORES = 8
HL = H // NCORES            # heads per core = 2
HDL = HL * HD               # local head dims = 256
T = B * S                   # 4096 tokens (b-major)
TCB = S // NCORES           # 256 tokens per core per batch after AllToAll
SCALE = float(1.0 / np.sqrt(HD))
BF = mybir.dt.bfloat16
F32 = mybir.dt.float32
AF = mybir.ActivationFunctionType


def build():
    nc = bacc.Bacc(None)
    xT = nc.declare_dram_parameter("xT", [D, T], BF, isOutput=False)
    # weights host-pretiled: [128, kb * out_cols] with kb-major columns
    wq = nc.declare_dram_parameter("wq", [128, 16 * HDL], BF, isOutput=False)
    wk = nc.declare_dram_parameter("wk", [128, 16 * HDL], BF, isOutput=False)
    wv = nc.declare_dram_parameter("wv", [128, 16 * HDL], BF, isOutput=False)
    wo = nc.declare_dram_parameter("wo", [128, 16 * D], BF, isOutput=False)
    cosT = nc.declare_dram_parameter("cosT", [HD, S], BF, isOutput=False)
    sinmT = nc.declare_dram_parameter("sinmT", [HD, S], BF, isOutput=False)
    maskT = nc.declare_dram_parameter("maskT", [128, 128], BF, isOutput=False)
    out = nc.declare_dram_parameter("out", [2 * TCB, D], F32, isOutput=True)

    with tile.TileContext(nc) as tc:
        with tc.tile_pool(name="persist", bufs=1) as persist, \
             tc.tile_pool(name="dram", bufs=1, space="DRAM") as dram, \
             tc.tile_pool(name="psA", bufs=3, space="PSUM") as psA, \
             tc.tile_pool(name="psO", bufs=2, space="PSUM") as psO, \
             tc.tile_pool(name="psD", bufs=2, space="PSUM") as psD, \
             tc.tile_pool(name="psP", bufs=1, space="PSUM") as psP, \
             tc.tile_pool(name="expp", bufs=6) as expp, \
             tc.tile_pool(name="yp", bufs=3) as yp, \
             tc.tile_pool(name="recp", bufs=2) as recp:

            cos_sb = persist.tile([HD, S], BF, tag="cos")
            sinm_sb = persist.tile([HD, S], BF, tag="sinm")
            mask_sb = persist.tile([128, 128], BF, tag="mask")
            ones_sb = persist.tile([128, 128], BF, tag="ones")
            qT_sb = [persist.tile([128, T], BF, name=f"qT{h}", tag=f"qT{h}") for h in range(HL)]
            kT_sb = [persist.tile([128, T], BF, name=f"kT{h}", tag=f"kT{h}") for h in range(HL)]
            v_sb = [persist.tile([128, HDL], BF, name=f"v{i}", tag=f"v{i}") for i in range(T // 128)]
            # per-(batch, token-half) AllToAll bounce buffers
            a2a_in = [[dram.tile([NCORES, HDL, 128], BF, name=f"a2a_in{b}_{hf}", tag=f"a2a_in{b}_{hf}")
                       for hf in range(2)] for b in range(B)]
            a2a_out = [[dram.tile([NCORES, HDL, 128], BF, name=f"a2a_out{b}_{hf}", tag=f"a2a_out{b}_{hf}")
                        for hf in range(2)] for b in range(B)]

            # ----- phase 1: QKV projections + RoPE -----
            with tc.tile_pool(name="xp", bufs=20) as xp, \
                 tc.tile_pool(name="wp", bufs=1) as wp, \
                 tc.tile_pool(name="tmp", bufs=4) as tmpp:
                # batch-0 activations first: they gate the first matmul
                xt = {}
                for th in range(2):
                    xt[th] = [xp.tile([128, 2048], BF, name=f"xt{th}_{i}", tag="x")
                              for i in range(16)]
                for kb in range(16):
                    eng = nc.sync if kb % 2 == 0 else nc.scalar
                    eng.dma_start(
                        xt[0][kb][:],
                        xT[kb * 128:(kb + 1) * 128, 0:2048])
                # weights + tables on the gpsimd queue, off the critical path
                w_sb = {}
                for name, wparam in (("q", wq), ("k", wk), ("v", wv)):
                    wt = wp.tile([128, 16 * HDL], BF, name=f"w{name}", tag=f"w{name}")
                    for qd in range(4):
                        nc.gpsimd.dma_start(
                            wt[:, qd * 1024:(qd + 1) * 1024],
                            wparam[:, qd * 1024:(qd + 1) * 1024])
                    w_sb[name] = wt
                nc.gpsimd.dma_start(cos_sb[:], cosT[:, :])
                nc.gpsimd.dma_start(sinm_sb[:], sinmT[:, :])
                nc.gpsimd.dma_start(mask_sb[:], maskT[:, :])
                nc.vector.memset(ones_sb[:], 1.0)
                for kb in range(16):
                    eng = nc.sync if kb % 2 == 0 else nc.scalar
                    eng.dma_start(
                        xt[1][kb][:],
                        xT[kb * 128:(kb + 1) * 128, 2048:4096])

                for th in range(2):  # token half == batch index
                    # Q, K in d-major layout with fused RoPE on eviction
                    for name, dest in (("q", qT_sb), ("k", kT_sb)):
                        for m in range(HL):
                            for n in range(4):
                                ps = psA.tile([128, 512], F32, tag="a")
                                for kb in range(16):
                                    nc.tensor.matmul(
                                        ps[:],
                                        w_sb[name][:, kb * HDL + m * 128: kb * HDL + (m + 1) * 128],
                                        xt[th][kb][:, n * 512:(n + 1) * 512],
                                        start=(kb == 0), stop=(kb == 15))
                                t1 = tmpp.tile([128, 512], F32, tag="t1")
                                t2 = tmpp.tile([128, 512], F32, tag="t2")
                                tg = slice(n * 512, (n + 1) * 512)
                                nc.vector.tensor_mul(t1[:], ps[:], cos_sb[:, tg])
                                nc.vector.tensor_mul(t2[0:64, :], ps[64:128, :], sinm_sb[0:64, tg])
                                nc.vector.tensor_mul(t2[64:128, :], ps[0:64, :], sinm_sb[64:128, tg])
                                dst = dest[m][:, th * 2048 + n * 512: th * 2048 + (n + 1) * 512]
                                nc.vector.tensor_add(dst, t1[:], t2[:])
                    # V in token-major layout
                    for tm in range(16):
                        psv = psA.tile([128, 512], F32, tag="a")
                        for kb in range(16):
                            nc.tensor.matmul(
                                psv[:, 0:HDL],
                                xt[th][kb][:, tm * 128:(tm + 1) * 128],
                                w_sb["v"][:, kb * HDL:(kb + 1) * HDL],
                                start=(kb == 0), stop=(kb == 15))
                        nc.scalar.copy(v_sb[th * 16 + tm][:], psv[:, 0:HDL])

            with tc.tile_pool(name="wop", bufs=1) as wop, \
                 tc.tile_pool(name="ytp", bufs=1) as ytp, \
                 tc.tile_pool(name="outp", bufs=3) as outp:
                # wo lands in the space freed by phase 1 and overlaps attention
                wo_sb = wop.tile([128, 16 * D], BF, tag="wo")
                for half in range(2):
                    nc.scalar.dma_start(
                        wo_sb[:, half * 8 * D:(half + 1) * 8 * D],
                        wo[:, half * 8 * D:(half + 1) * 8 * D])

                def out_proj(b, hf):
                    """project my 128-token chunk of (batch b, half hf)"""
                    yt = ytp.tile([128, 16 * 128], BF, name=f"yt{b}_{hf}", tag=f"yt{b}_{hf}")
                    nc.sync.dma_start(
                        yt[:],
                        a2a_out[b][hf].rearrange("r (h d) t -> d r h t", h=2))
                    for ncol in range(4):
                        ps = psP.tile([128, 512], F32, tag="p")
                        for kb in range(16):
                            nc.tensor.matmul(
                                ps[:],
                                yt[:, kb * 128:(kb + 1) * 128],
                                wo_sb[:, kb * D + ncol * 512: kb * D + (ncol + 1) * 512],
                                start=(kb == 0), stop=(kb == 15))
                        o = outp.tile([128, 512], F32, tag="ot")
                        nc.scalar.copy(o[:], ps[:])
                        nc.sync.dma_start(
                            out[b * TCB + hf * 128: b * TCB + hf * 128 + 128,
                                ncol * 512:(ncol + 1) * 512],
                            o[:])

                # ----- phase 2: causal attention in [k, q] layout -----
                for b in range(B):
                    for qsb in range(4):
                        for h in range(HL):
                            out_ps = psO.tile([128, 512], F32, tag="o")
                            den_ps = psD.tile([128, 512], F32, tag="d")
                            nkb = (qsb + 1) * 4
                            for kb in range(nkb):
                                j = kb - qsb * 4  # >=0 on the causal diagonal band
                                c0 = max(j, 0) * 128  # first valid q column
                                lg = psA.tile([128, 512], F32, tag="a")
                                nc.tensor.matmul(
                                    lg[:, c0:512],
                                    kT_sb[h][:, b * S + kb * 128: b * S + (kb + 1) * 128],
                                    qT_sb[h][:, b * S + qsb * 512 + c0: b * S + (qsb + 1) * 512],
                                    start=True, stop=True)
                                pe = expp.tile([128, 512], BF, tag="e")
                                nc.scalar.activation(pe[:, c0:512], lg[:, c0:512],
                                                     AF.Exp, scale=SCALE)
                                if j >= 0:
                                    nc.vector.tensor_mul(
                                        pe[:, c0:c0 + 128], pe[:, c0:c0 + 128], mask_sb[:])
                                nc.tensor.matmul(
                                    out_ps[:, c0:512],
                                    v_sb[b * 16 + kb][:, h * 128:(h + 1) * 128],
                                    pe[:, c0:512],
                                    start=(kb == 0), stop=(kb == nkb - 1))
                                nc.tensor.matmul(
                                    den_ps[:, c0:512], ones_sb[:], pe[:, c0:512],
                                    start=(kb == 0), stop=(kb == nkb - 1))
                            rec = recp.tile([128, 512], F32, tag="r")
                            nc.vector.reciprocal_approx_fast(rec[:], den_ps[:])
                            y = yp.tile([128, 512], BF, tag="y")
                            nc.vector.tensor_mul(y[:], out_ps[:], rec[:])
                            for jj in range(4):
                                nc.sync.dma_start(
                                    a2a_in[b][qsb // 2][(qsb % 2) * 4 + jj,
                                                        h * 128:(h + 1) * 128, :],
                                    y[:, jj * 128:(jj + 1) * 128])
                        if qsb % 2 == 1:
                            nc.gpsimd.collective_compute(
                                "AllToAll", mybir.AluOpType.bypass,
                                ins=[a2a_in[b][qsb // 2].opt()],
                                outs=[a2a_out[b][qsb // 2].opt()],
                                replica_groups=[list(range(NCORES))])
                # emitted last so they gap-fill the attention phase once
                # their AllToAll lands
                out_proj(0, 0)
                out_proj(0, 1)
                out_proj(1, 0)
                out_proj(1, 1)
    nc.finalize()
    return nc


_CACHE = {}


def _get_nc():
    if "nc" not in _CACHE:
        _CACHE["nc"] = build()
    return _CACHE["nc"]


def _pretile(w, cols):
    """[2048, cols] -> [128, 16*cols], kb-major along columns."""
    return np.ascontiguousarray(
        w.reshape(16, 128, cols).transpose(1, 0, 2).reshape(128, 16 * cols)
    ).astype(BF16)


def _prep_in_maps(x, freq_cos, freq_sin, wq, wk, wv, wo):
    xTa = np.asarray(x, np.float32).reshape(T, D).T.astype(BF16)
    cos_t = np.asarray(freq_cos, np.float32).T  # [64, S]
    sin_t = np.asarray(freq_sin, np.float32).T
    cosT = np.concatenate([cos_t, cos_t], 0).astype(BF16)
    sinmT = np.concatenate([-sin_t, sin_t], 0).astype(BF16)
    kk = np.arange(128)[:, None]
    qq = np.arange(128)[None, :]
    maskTa = (qq >= kk).astype(BF16)
    wo_t = _pretile(np.asarray(wo, np.float32), D)
    wq = np.asarray(wq, np.float32)
    wk = np.asarray(wk, np.float32)
    wv = np.asarray(wv, np.float32)
    in_maps = []
    for c in range(NCORES):
        sl = slice(c * HDL, (c + 1) * HDL)
        in_maps.append({
            "xT": xTa,
            "wq": _pretile(wq[:, sl], HDL),
            "wk": _pretile(wk[:, sl], HDL),
            "wv": _pretile(wv[:, sl], HDL),
            "wo": wo_t,
            "cosT": cosT,
            "sinmT": sinmT,
            "maskT": maskTa,
        })
    return in_maps


def kernel(x, freq_cos, freq_sin, wq, wk, wv, wo, _trace=False):
    nc = _get_nc()
    in_maps = _prep_in_maps(x, freq_cos, freq_sin, wq, wk, wv, wo)
    res = run_bass_kernel_spmd(
        nc, in_maps, core_ids=list(range(NCORES)), trace=_trace)
    # core c holds tokens [hf*1024 + c*128) of each (batch, half)
    full = np.empty((B, S, D), np.float32)
    for c in range(NCORES):
        o = np.asarray(res.results[c]["out"], np.float32)
        for b in range(B):
            for hf in range(2):
                full[b, hf * 1024 + c * 128: hf * 1024 + (c + 1) * 128] = \
                    o[b * TCB + hf * 128: b * TCB + hf * 128 + 128]
    if _trace:
        return full, res
    return full


# revision 11
# speedup vs baseline: 1.0593x; 1.0593x over previous
"""Dense causal attention block (QKV proj + RoPE + causal attention + out proj)
distributed over 8 TRN2 NeuronCores.

Sharding: tensor-parallel over heads (2 heads/core). Each core computes
q/k/v for its heads (q/k d-major so QK^T needs no transposes, v
token-major), causal attention in logits-transposed [k, q] layout
(softmax denominator via an all-ones stationary matmul, no max
subtraction -- logits are O(1) -- normalization folded into the PSUM
eviction), then one AllToAll per batch converts head-sharding to
token-sharding so the output projection overlaps the other batch's
attention. Host assembles the 16 token chunks.

All matmuls run in bf16 (f32 PSUM accumulate); host pre-casts and
pre-tiles the weights so every DMA is partition-contiguous.
"""
import numpy as np
import ml_dtypes

import concourse.bass as bass
import concourse.mybir as mybir
from concourse import tile, bacc
from concourse.bass_utils import run_bass_kernel_spmd

BF16 = ml_dtypes.bfloat16
B, S, D = 2, 2048, 2048
H, HD = 16, 128
NCORES = 8
HL = H // NCORES            # heads per core = 2
HDL = HL * HD               # local head dims = 256
T = B * S                   # 4096 tokens (b-major)
TCB = S // NCORES           # 256 tokens per core per batch after AllToAll
SCALE = float(1.0 / np.sqrt(HD))
BF = mybir.dt.bfloat16
F32 = mybir.dt.float32
AF = mybir.ActivationFunctionType


def build():
    nc = bacc.Bacc(None)
    xT = nc.declare_dram_parameter("xT", [D, T], BF, isOutput=False)
    # weights host-pretiled: [128, kb * out_cols] with kb-major columns
    wq = nc.declare_dram_parameter("wq", [128, 16 * HDL], BF, isOutput=False)
    wk = nc.declare_dram_parameter("wk", [128, 16 * HDL], BF, isOutput=False)
    wv = nc.declare_dram_parameter("wv", [128, 16 * HDL], BF, isOutput=False)
    wo = nc.declare_dram_parameter("wo", [128, 16 * D], BF, isOutput=False)
    cosT = nc.declare_dram_parameter("cosT", [HD, S], BF, isOutput=False)
    sinmT = nc.declare_dram_parameter("sinmT", [HD, S], BF, isOutput=False)
    maskT = nc.declare_dram_parameter("maskT", [128, 128], BF, isOutput=False)
    out = nc.declare_dram_parameter("out", [2 * TCB, D], F32, isOutput=True)

    with tile.TileContext(nc) as tc:
        with tc.tile_pool(name="persist", bufs=1) as persist, \
             tc.tile_pool(name="dram", bufs=1, space="DRAM") as dram, \
             tc.tile_pool(name="psA", bufs=3, space="PSUM") as psA, \
             tc.tile_pool(name="psO", bufs=2, space="PSUM") as psO, \
             tc.tile_pool(name="psD", bufs=2, space="PSUM") as psD, \
             tc.tile_pool(name="psP", bufs=1, space="PSUM") as psP, \
             tc.tile_pool(name="expp", bufs=6) as expp, \
             tc.tile_pool(name="yp", bufs=3) as yp, \
             tc.tile_pool(name="recp", bufs=2) as recp:

            cos_sb = persist.tile([HD, S], BF, tag="cos")
            sinm_sb = persist.tile([HD, S], BF, tag="sinm")
            mask_sb = persist.tile([128, 128], BF, tag="mask")
            ones_sb = persist.tile([128, 128], BF, tag="ones")
            qT_sb = [persist.tile([128, T], BF, name=f"qT{h}", tag=f"qT{h}") for h in range(HL)]
            kT_sb = [persist.tile([128, T], BF, name=f"kT{h}", tag=f"kT{h}") for h in range(HL)]
            v_sb = [persist.tile([128, HDL], BF, name=f"v{i}", tag=f"v{i}") for i in range(T // 128)]
            # per-(batch, token-half) AllToAll bounce buffers
            a2a_in = [[dram.tile([NCORES, HDL, 128], BF, name=f"a2a_in{b}_{hf}", tag=f"a2a_in{b}_{hf}")
                       for hf in range(2)] for b in range(B)]
            a2a_out = [[dram.tile([NCORES, HDL, 128], BF, name=f"a2a_out{b}_{hf}", tag=f"a2a_out{b}_{hf}")
                        for hf in range(2)] for b in range(B)]

            # warmup collective: absorbs the ~30us first-collective ncfw
            # setup cost while phase 1 computes
            dum_in = dram.tile([NCORES, 1, 128], BF, tag="dum_in")
            dum_out = dram.tile([NCORES, 1, 128], BF, tag="dum_out")
            nc.gpsimd.collective_compute(
                "AllToAll", mybir.AluOpType.bypass,
                ins=[dum_in.opt()], outs=[dum_out.opt()],
                replica_groups=[list(range(NCORES))])

            # ----- phase 1: QKV projections + RoPE -----
            with tc.tile_pool(name="xp", bufs=20) as xp, \
                 tc.tile_pool(name="wp", bufs=1) as wp, \
                 tc.tile_pool(name="tmp", bufs=4) as tmpp:
                # batch-0 activations first: they gate the first matmul
                xt = {}
                for th in range(2):
                    xt[th] = [xp.tile([128, 2048], BF, name=f"xt{th}_{i}", tag="x")
                              for i in range(16)]
                for kb in range(16):
                    eng = nc.sync if kb % 2 == 0 else nc.scalar
                    eng.dma_start(
                        xt[0][kb][:],
                        xT[kb * 128:(kb + 1) * 128, 0:2048])
                # weights + tables on the gpsimd queue, off the critical path
                w_sb = {}
                for name, wparam in (("q", wq), ("k", wk), ("v", wv)):
                    wt = wp.tile([128, 16 * HDL], BF, name=f"w{name}", tag=f"w{name}")
                    for qd in range(4):
                        nc.gpsimd.dma_start(
                            wt[:, qd * 1024:(qd + 1) * 1024],
                            wparam[:, qd * 1024:(qd + 1) * 1024])
                    w_sb[name] = wt
                nc.gpsimd.dma_start(cos_sb[:], cosT[:, :])
                nc.gpsimd.dma_start(sinm_sb[:], sinmT[:, :])
                nc.gpsimd.dma_start(mask_sb[:], maskT[:, :])
                nc.vector.memset(ones_sb[:], 1.0)
                for kb in range(16):
                    eng = nc.sync if kb % 2 == 0 else nc.scalar
                    eng.dma_start(
                        xt[1][kb][:],
                        xT[kb * 128:(kb + 1) * 128, 2048:4096])

                for th in range(2):  # token half == batch index
                    # Q, K in d-major layout with fused RoPE on eviction
                    for name, dest in (("q", qT_sb), ("k", kT_sb)):
                        for m in range(HL):
                            for n in range(4):
                                ps = psA.tile([128, 512], F32, tag="a")
                                for kb in range(16):
                                    nc.tensor.matmul(
                                        ps[:],
                                        w_sb[name][:, kb * HDL + m * 128: kb * HDL + (m + 1) * 128],
                                        xt[th][kb][:, n * 512:(n + 1) * 512],
                                        start=(kb == 0), stop=(kb == 15))
                                t1 = tmpp.tile([128, 512], F32, tag="t1")
                                t2 = tmpp.tile([128, 512], F32, tag="t2")
                                tg = slice(n * 512, (n + 1) * 512)
                                nc.vector.tensor_mul(t1[:], ps[:], cos_sb[:, tg])
                                nc.vector.tensor_mul(t2[0:64, :], ps[64:128, :], sinm_sb[0:64, tg])
                                nc.vector.tensor_mul(t2[64:128, :], ps[0:64, :], sinm_sb[64:128, tg])
                                dst = dest[m][:, th * 2048 + n * 512: th * 2048 + (n + 1) * 512]
                                nc.vector.tensor_add(dst, t1[:], t2[:])
                    # V in token-major layout
                    for tm in range(16):
                        psv = psA.tile([128, 512], F32, tag="a")
                        for kb in range(16):
                            nc.tensor.matmul(
                                psv[:, 0:HDL],
                                xt[th][kb][:, tm * 128:(tm + 1) * 128],
                                w_sb["v"][:, kb * HDL:(kb + 1) * HDL],
                                start=(kb == 0), stop=(kb == 15))
                        nc.scalar.copy(v_sb[th * 16 + tm][:], psv[:, 0:HDL])

            with tc.tile_pool(name="wop", bufs=1) as wop, \
                 tc.tile_pool(name="ytp", bufs=1) as ytp, \
                 tc.tile_pool(name="outp", bufs=3) as outp:
                # wo lands in the space freed by phase 1 and overlaps attention
                wo_sb = wop.tile([128, 16 * D], BF, tag="wo")
                for half in range(2):
                    nc.scalar.dma_start(
                        wo_sb[:, half * 8 * D:(half + 1) * 8 * D],
                        wo[:, half * 8 * D:(half + 1) * 8 * D])

                def out_proj(b, hf):
                    """project my 128-token chunk of (batch b, half hf)"""
                    yt = ytp.tile([128, 16 * 128], BF, name=f"yt{b}_{hf}", tag=f"yt{b}_{hf}")
                    nc.gpsimd.dma_start(
                        yt[:],
                        a2a_out[b][hf].rearrange("r (h d) t -> d r h t", h=2))
                    for ncol in range(4):
                        ps = psP.tile([128, 512], F32, tag="p")
                        for kb in range(16):
                            nc.tensor.matmul(
                                ps[:],
                                yt[:, kb * 128:(kb + 1) * 128],
                                wo_sb[:, kb * D + ncol * 512: kb * D + (ncol + 1) * 512],
                                start=(kb == 0), stop=(kb == 15))
                        o = outp.tile([128, 512], F32, tag="ot")
                        nc.scalar.copy(o[:], ps[:])
                        nc.gpsimd.dma_start(
                            out[b * TCB + hf * 128: b * TCB + hf * 128 + 128,
                                ncol * 512:(ncol + 1) * 512],
                            o[:])

                # ----- phase 2: causal attention in [k, q] layout -----
                for b in range(B):
                    for qsb in range(4):
                        for h in range(HL):
                            out_ps = psO.tile([128, 512], F32, tag="o")
                            den_ps = psD.tile([128, 512], F32, tag="d")
                            nkb = (qsb + 1) * 4
                            for kb in range(nkb):
                                j = kb - qsb * 4  # >=0 on the causal diagonal band
                                c0 = max(j, 0) * 128  # first valid q column
                                lg = psA.tile([128, 512], F32, tag="a")
                                nc.tensor.matmul(
                                    lg[:, c0:512],
                                    kT_sb[h][:, b * S + kb * 128: b * S + (kb + 1) * 128],
                                    qT_sb[h][:, b * S + qsb * 512 + c0: b * S + (qsb + 1) * 512],
                                    start=True, stop=True)
                                pe = expp.tile([128, 512], BF, tag="e")
                                nc.scalar.activation(pe[:, c0:512], lg[:, c0:512],
                                                     AF.Exp, scale=SCALE)
                                if j >= 0:
                                    nc.vector.tensor_mul(
                                        pe[:, c0:c0 + 128], pe[:, c0:c0 + 128], mask_sb[:])
                                nc.tensor.matmul(
                                    out_ps[:, c0:512],
                                    v_sb[b * 16 + kb][:, h * 128:(h + 1) * 128],
                                    pe[:, c0:512],
                                    start=(kb == 0), stop=(kb == nkb - 1))
                                nc.tensor.matmul(
                                    den_ps[:, c0:512], ones_sb[:], pe[:, c0:512],
                                    start=(kb == 0), stop=(kb == nkb - 1))
                            rec = recp.tile([128, 512], F32, tag="r")
                            nc.vector.reciprocal_approx_fast(rec[:], den_ps[:])
                            y = yp.tile([128, 512], BF, tag="y")
                            nc.vector.tensor_mul(y[:], out_ps[:], rec[:])
                            for jj in range(4):
                                nc.sync.dma_start(
                                    a2a_in[b][qsb // 2][(qsb % 2) * 4 + jj,
                                                        h * 128:(h + 1) * 128, :],
                                    y[:, jj * 128:(jj + 1) * 128])
                        if qsb % 2 == 1:
                            nc.gpsimd.collective_compute(
                                "AllToAll", mybir.AluOpType.bypass,
                                ins=[a2a_in[b][qsb // 2].opt()],
                                outs=[a2a_out[b][qsb // 2].opt()],
                                replica_groups=[list(range(NCORES))])
                # emitted last so they gap-fill the attention phase once
                # their AllToAll lands
                out_proj(0, 0)
                out_proj(0, 1)
                out_proj(1, 0)
                out_proj(1, 1)
    nc.finalize()
    return nc


_CACHE = {}


def _get_nc():
    if "nc" not in _CACHE:
        _CACHE["nc"] = build()
    return _CACHE["nc"]


def _pretile(w, cols):
    """[2048, cols] -> [128, 16*cols], kb-major along columns."""
    return np.ascontiguousarray(
        w.reshape(16, 128, cols).transpose(1, 0, 2).reshape(128, 16 * cols)
    ).astype(BF16)


def _prep_in_maps(x, freq_cos, freq_sin, wq, wk, wv, wo):
    xTa = np.asarray(x, np.float32).reshape(T, D).T.astype(BF16)
    cos_t = np.asarray(freq_cos, np.float32).T  # [64, S]
    sin_t = np.asarray(freq_sin, np.float32).T
    cosT = np.concatenate([cos_t, cos_t], 0).astype(BF16)
    sinmT = np.concatenate([-sin_t, sin_t], 0).astype(BF16)
    kk = np.arange(128)[:, None]
    qq = np.arange(128)[None, :]
    maskTa = (qq >= kk).astype(BF16)
    wo_t = _pretile(np.asarray(wo, np.float32), D)
    wq = np.asarray(wq, np.float32)
    wk = np.asarray(wk, np.float32)
    wv = np.asarray(wv, np.float32)
    in_maps = []
    for c in range(NCORES):
        sl = slice(c * HDL, (c + 1) * HDL)
        in_maps.append({
            "xT": xTa,
            "wq": _pretile(wq[:, sl], HDL),
            "wk": _pretile(wk[:, sl], HDL),
            "wv": _pretile(wv[:, sl], HDL),
            "wo": wo_t,
            "cosT": cosT,
            "sinmT": sinmT,
            "maskT": maskTa,
        })
    return in_maps


def kernel(x, freq_cos, freq_sin, wq, wk, wv, wo, _trace=False):
    nc = _get_nc()
    in_maps = _prep_in_maps(x, freq_cos, freq_sin, wq, wk, wv, wo)
    res = run_bass_kernel_spmd(
        nc, in_maps, core_ids=list(range(NCORES)), trace=_trace)
    # core c holds tokens [hf*1024 + c*128) of each (batch, half)
    full = np.empty((B, S, D), np.float32)
    for c in range(NCORES):
        o = np.asarray(res.results[c]["out"], np.float32)
        for b in range(B):
            for hf in range(2):
                full[b, hf * 1024 + c * 128: hf * 1024 + (c + 1) * 128] = \
                    o[b * TCB + hf * 128: b * TCB + hf * 128 + 128]
    if _trace:
        return full, res
    return full


# revision 12
# speedup vs baseline: 1.1238x; 1.0610x over previous
"""Dense causal attention block (QKV proj + RoPE + causal attention + out proj)
distributed over 8 TRN2 NeuronCores.

Sharding: tensor-parallel over heads (2 heads/core). Each core computes
q/k/v for its heads (q/k d-major so QK^T needs no transposes, v
token-major), causal attention in logits-transposed [k, q] layout
(softmax denominator via an all-ones stationary matmul, no max
subtraction -- logits are O(1) -- normalization folded into the PSUM
eviction), then one AllToAll per batch converts head-sharding to
token-sharding so the output projection overlaps the other batch's
attention. Host assembles the 16 token chunks.

All matmuls run in bf16 (f32 PSUM accumulate); host pre-casts and
pre-tiles the weights so every DMA is partition-contiguous.
"""
import numpy as np
import ml_dtypes

import concourse.bass as bass
import concourse.mybir as mybir
from concourse import tile, bacc
from concourse.bass_utils import run_bass_kernel_spmd

BF16 = ml_dtypes.bfloat16
B, S, D = 2, 2048, 2048
H, HD = 16, 128
NCORES = 8
HL = H // NCORES            # heads per core = 2
HDL = HL * HD               # local head dims = 256
T = B * S                   # 4096 tokens (b-major)
TCB = S // NCORES           # 256 tokens per core per batch after AllToAll
SCALE = float(1.0 / np.sqrt(HD))
BF = mybir.dt.bfloat16
F32 = mybir.dt.float32
AF = mybir.ActivationFunctionType


def build():
    nc = bacc.Bacc(None)
    xT = nc.declare_dram_parameter("xT", [D, T], BF, isOutput=False)
    # weights host-pretiled: [128, kb * out_cols] with kb-major columns
    wq = nc.declare_dram_parameter("wq", [128, 16 * HDL], BF, isOutput=False)
    wk = nc.declare_dram_parameter("wk", [128, 16 * HDL], BF, isOutput=False)
    wv = nc.declare_dram_parameter("wv", [128, 16 * HDL], BF, isOutput=False)
    wo = nc.declare_dram_parameter("wo", [128, 16 * D], BF, isOutput=False)
    cosT = nc.declare_dram_parameter("cosT", [HD, S], BF, isOutput=False)
    sinmT = nc.declare_dram_parameter("sinmT", [HD, S], BF, isOutput=False)
    maskT = nc.declare_dram_parameter("maskT", [128, 128], BF, isOutput=False)
    out = nc.declare_dram_parameter("out", [2 * TCB, D], F32, isOutput=True)

    with tile.TileContext(nc) as tc:
        with tc.tile_pool(name="persist", bufs=1) as persist, \
             tc.tile_pool(name="dram", bufs=1, space="DRAM") as dram, \
             tc.tile_pool(name="psA", bufs=3, space="PSUM") as psA, \
             tc.tile_pool(name="psO", bufs=2, space="PSUM") as psO, \
             tc.tile_pool(name="psD", bufs=2, space="PSUM") as psD, \
             tc.tile_pool(name="psP", bufs=1, space="PSUM") as psP, \
             tc.tile_pool(name="expp", bufs=8) as expp, \
             tc.tile_pool(name="yp", bufs=3) as yp, \
             tc.tile_pool(name="recp", bufs=2) as recp:

            cos_sb = persist.tile([HD, S], BF, tag="cos")
            sinm_sb = persist.tile([HD, S], BF, tag="sinm")
            mask_sb = persist.tile([128, 128], BF, tag="mask")
            ones_sb = persist.tile([128, 128], BF, tag="ones")
            qT_sb = [persist.tile([128, T], BF, name=f"qT{h}", tag=f"qT{h}") for h in range(HL)]
            kT_sb = [persist.tile([128, T], BF, name=f"kT{h}", tag=f"kT{h}") for h in range(HL)]
            v_sb = [persist.tile([128, HDL], BF, name=f"v{i}", tag=f"v{i}") for i in range(T // 128)]
            # per-(batch, token-half) AllToAll bounce buffers
            a2a_in = [[dram.tile([NCORES, HDL, 128], BF, name=f"a2a_in{b}_{hf}", tag=f"a2a_in{b}_{hf}")
                       for hf in range(2)] for b in range(B)]
            a2a_out = [[dram.tile([NCORES, HDL, 128], BF, name=f"a2a_out{b}_{hf}", tag=f"a2a_out{b}_{hf}")
                        for hf in range(2)] for b in range(B)]

            # warmup collective: absorbs the ~30us first-collective ncfw
            # setup cost while phase 1 computes
            dum_in = dram.tile([NCORES, 1, 128], BF, tag="dum_in")
            dum_out = dram.tile([NCORES, 1, 128], BF, tag="dum_out")
            nc.gpsimd.collective_compute(
                "AllToAll", mybir.AluOpType.bypass,
                ins=[dum_in.opt()], outs=[dum_out.opt()],
                replica_groups=[list(range(NCORES))])

            # ----- phase 1: QKV projections + RoPE -----
            with tc.tile_pool(name="xp", bufs=36) as xp, \
                 tc.tile_pool(name="wp", bufs=1) as wp, \
                 tc.tile_pool(name="tmp", bufs=4) as tmpp:
                # x arrives as [kb, n-chunk] tiles of [128, 512] so the first
                # projection PSUM only waits on 2 MB, not the whole half
                xt = {}
                for th in range(2):
                    for n in range(4):
                        xt[th, n] = [xp.tile([128, 512], BF, name=f"xt{th}_{n}_{i}", tag="x")
                                     for i in range(16)]
                w_sb = {}
                wdma_emitted = False

                def emit_wdmas():
                    for name, wparam in (("q", wq), ("k", wk), ("v", wv)):
                        wt = wp.tile([128, 16 * HDL], BF, name=f"w{name}", tag=f"w{name}")
                        for qd in range(4):
                            nc.gpsimd.dma_start(
                                wt[:, qd * 1024:(qd + 1) * 1024],
                                wparam[:, qd * 1024:(qd + 1) * 1024])
                        w_sb[name] = wt
                    nc.gpsimd.dma_start(cos_sb[:], cosT[:, :])
                    nc.gpsimd.dma_start(sinm_sb[:], sinmT[:, :])
                    nc.gpsimd.dma_start(mask_sb[:], maskT[:, :])
                    nc.vector.memset(ones_sb[:], 1.0)

                for th in range(2):
                    for n in range(4):
                        for kb in range(16):
                            eng = nc.sync if kb % 2 == 0 else nc.scalar
                            eng.dma_start(
                                xt[th, n][kb][:],
                                xT[kb * 128:(kb + 1) * 128,
                                   th * 2048 + n * 512: th * 2048 + (n + 1) * 512])
                        if not wdma_emitted:
                            wdma_emitted = True
                            emit_wdmas()

                for th in range(2):  # token half == batch index
                    for n in range(4):
                        # Q, K in d-major layout with fused RoPE on eviction
                        for name, dest in (("q", qT_sb), ("k", kT_sb)):
                            for m in range(HL):
                                ps = psA.tile([128, 512], F32, tag="a")
                                for kb in range(16):
                                    nc.tensor.matmul(
                                        ps[:],
                                        w_sb[name][:, kb * HDL + m * 128: kb * HDL + (m + 1) * 128],
                                        xt[th, n][kb][:],
                                        start=(kb == 0), stop=(kb == 15))
                                t1 = tmpp.tile([128, 512], F32, tag="t1")
                                t2 = tmpp.tile([128, 512], F32, tag="t2")
                                tg = slice(n * 512, (n + 1) * 512)
                                nc.vector.tensor_mul(t1[:], ps[:], cos_sb[:, tg])
                                nc.vector.tensor_mul(t2[0:64, :], ps[64:128, :], sinm_sb[0:64, tg])
                                nc.vector.tensor_mul(t2[64:128, :], ps[0:64, :], sinm_sb[64:128, tg])
                                dst = dest[m][:, th * 2048 + n * 512: th * 2048 + (n + 1) * 512]
                                nc.vector.tensor_add(dst, t1[:], t2[:])
                        # V in token-major layout
                        for tmi in range(4):
                            tm = n * 4 + tmi
                            psv = psA.tile([128, 512], F32, tag="a")
                            for kb in range(16):
                                nc.tensor.matmul(
                                    psv[:, 0:HDL],
                                    xt[th, n][kb][:, tmi * 128:(tmi + 1) * 128],
                                    w_sb["v"][:, kb * HDL:(kb + 1) * HDL],
                                    start=(kb == 0), stop=(kb == 15))
                            nc.scalar.copy(v_sb[th * 16 + tm][:], psv[:, 0:HDL])

            with tc.tile_pool(name="wop", bufs=1) as wop, \
                 tc.tile_pool(name="ytp", bufs=1) as ytp, \
                 tc.tile_pool(name="outp", bufs=3) as outp:
                # wo lands in the space freed by phase 1 and overlaps attention
                wo_sb = wop.tile([128, 16 * D], BF, tag="wo")
                for half in range(2):
                    nc.scalar.dma_start(
                        wo_sb[:, half * 8 * D:(half + 1) * 8 * D],
                        wo[:, half * 8 * D:(half + 1) * 8 * D])

                def out_proj(b, hf):
                    """project my 128-token chunk of (batch b, half hf)"""
                    yt = ytp.tile([128, 16 * 128], BF, name=f"yt{b}_{hf}", tag=f"yt{b}_{hf}")
                    nc.gpsimd.dma_start(
                        yt[:],
                        a2a_out[b][hf].rearrange("r (h d) t -> d r h t", h=2))
                    for ncol in range(4):
                        ps = psP.tile([128, 512], F32, tag="p")
                        for kb in range(16):
                            nc.tensor.matmul(
                                ps[:],
                                yt[:, kb * 128:(kb + 1) * 128],
                                wo_sb[:, kb * D + ncol * 512: kb * D + (ncol + 1) * 512],
                                start=(kb == 0), stop=(kb == 15))
                        o = outp.tile([128, 512], F32, tag="ot")
                        nc.scalar.copy(o[:], ps[:])
                        nc.gpsimd.dma_start(
                            out[b * TCB + hf * 128: b * TCB + hf * 128 + 128,
                                ncol * 512:(ncol + 1) * 512],
                            o[:])

                # ----- phase 2: causal attention in [k, q] layout -----
                for b in range(B):
                    for qsb in range(4):
                        for h in range(HL):
                            out_ps = psO.tile([128, 512], F32, tag="o")
                            den_ps = psD.tile([128, 512], F32, tag="d")
                            nkb = (qsb + 1) * 4
                            for kb in range(nkb):
                                j = kb - qsb * 4  # >=0 on the causal diagonal band
                                c0 = max(j, 0) * 128  # first valid q column
                                lg = psA.tile([128, 512], F32, tag="a")
                                nc.tensor.matmul(
                                    lg[:, c0:512],
                                    kT_sb[h][:, b * S + kb * 128: b * S + (kb + 1) * 128],
                                    qT_sb[h][:, b * S + qsb * 512 + c0: b * S + (qsb + 1) * 512],
                                    start=True, stop=True)
                                pe = expp.tile([128, 512], BF, tag="e")
                                nc.scalar.activation(pe[:, c0:512], lg[:, c0:512],
                                                     AF.Exp, scale=SCALE)
                                if j >= 0:
                                    nc.vector.tensor_mul(
                                        pe[:, c0:c0 + 128], pe[:, c0:c0 + 128], mask_sb[:])
                                nc.tensor.matmul(
                                    out_ps[:, c0:512],
                                    v_sb[b * 16 + kb][:, h * 128:(h + 1) * 128],
                                    pe[:, c0:512],
                                    start=(kb == 0), stop=(kb == nkb - 1))
                                nc.tensor.matmul(
                                    den_ps[:, c0:512], ones_sb[:], pe[:, c0:512],
                                    start=(kb == 0), stop=(kb == nkb - 1))
                            rec = recp.tile([128, 512], F32, tag="r")
                            nc.vector.reciprocal_approx_fast(rec[:], den_ps[:])
                            y = yp.tile([128, 512], BF, tag="y")
                            nc.vector.tensor_mul(y[:], out_ps[:], rec[:])
                            for jj in range(4):
                                nc.sync.dma_start(
                                    a2a_in[b][qsb // 2][(qsb % 2) * 4 + jj,
                                                        h * 128:(h + 1) * 128, :],
                                    y[:, jj * 128:(jj + 1) * 128])
                        if qsb % 2 == 1:
                            nc.gpsimd.collective_compute(
                                "AllToAll", mybir.AluOpType.bypass,
                                ins=[a2a_in[b][qsb // 2].opt()],
                                outs=[a2a_out[b][qsb // 2].opt()],
                                replica_groups=[list(range(NCORES))])
                # emitted last so they gap-fill the attention phase once
                # their AllToAll lands
                out_proj(0, 0)
                out_proj(0, 1)
                out_proj(1, 0)
                out_proj(1, 1)
    nc.finalize()
    return nc


_CACHE = {}


def _get_nc():
    if "nc" not in _CACHE:
        _CACHE["nc"] = build()
    return _CACHE["nc"]


def _pretile(w, cols):
    """[2048, cols] -> [128, 16*cols], kb-major along columns."""
    return np.ascontiguousarray(
        w.reshape(16, 128, cols).transpose(1, 0, 2).reshape(128, 16 * cols)
    ).astype(BF16)


def _prep_in_maps(x, freq_cos, freq_sin, wq, wk, wv, wo):
    xTa = np.asarray(x, np.float32).reshape(T, D).T.astype(BF16)
    cos_t = np.asarray(freq_cos, np.float32).T  # [64, S]
    sin_t = np.asarray(freq_sin, np.float32).T
    cosT = np.concatenate([cos_t, cos_t], 0).astype(BF16)
    sinmT = np.concatenate([-sin_t, sin_t], 0).astype(BF16)
    kk = np.arange(128)[:, None]
    qq = np.arange(128)[None, :]
    maskTa = (qq >= kk).astype(BF16)
    wo_t = _pretile(np.asarray(wo, np.float32), D)
    wq = np.asarray(wq, np.float32)
    wk = np.asarray(wk, np.float32)
    wv = np.asarray(wv, np.float32)
    in_maps = []
    for c in range(NCORES):
        sl = slice(c * HDL, (c + 1) * HDL)
        in_maps.append({
            "xT": xTa,
            "wq": _pretile(wq[:, sl], HDL),
            "wk": _pretile(wk[:, sl], HDL),
            "wv": _pretile(wv[:, sl], HDL),
            "wo": wo_t,
            "cosT": cosT,
            "sinmT": sinmT,
            "maskT": maskTa,
        })
    return in_maps


def kernel(x, freq_cos, freq_sin, wq, wk, wv, wo, _trace=False):
    nc = _get_nc()
    in_maps = _prep_in_maps(x, freq_cos, freq_sin, wq, wk, wv, wo)
    res = run_bass_kernel_spmd(
        nc, in_maps, core_ids=list(range(NCORES)), trace=_trace)
    # core c holds tokens [hf*1024 + c*128) of each (batch, half)
    full = np.empty((B, S, D), np.float32)
    for c in range(NCORES):
        o = np.asarray(res.results[c]["out"], np.float32)
        for b in range(B):
            for hf in range(2):
                full[b, hf * 1024 + c * 128: hf * 1024 + (c + 1) * 128] = \
                    o[b * TCB + hf * 128: b * TCB + hf * 128 + 128]
    if _trace:
        return full, res
    return full


# revision 13
# speedup vs baseline: 1.1476x; 1.0212x over previous
"""Dense causal attention block (QKV proj + RoPE + causal attention + out proj)
distributed over 8 TRN2 NeuronCores.

Sharding: tensor-parallel over heads (2 heads/core). Each core computes
q/k/v for its heads (q/k d-major so QK^T needs no transposes, v
token-major), causal attention in logits-transposed [k, q] layout
(softmax denominator via an all-ones stationary matmul, no max
subtraction -- logits are O(1) -- normalization folded into the PSUM
eviction), then one AllToAll per batch converts head-sharding to
token-sharding so the output projection overlaps the other batch's
attention. Host assembles the 16 token chunks.

All matmuls run in bf16 (f32 PSUM accumulate); host pre-casts and
pre-tiles the weights so every DMA is partition-contiguous.
"""
import numpy as np
import ml_dtypes

import concourse.bass as bass
import concourse.mybir as mybir
from concourse import tile, bacc
from concourse.bass_utils import run_bass_kernel_spmd

BF16 = ml_dtypes.bfloat16
B, S, D = 2, 2048, 2048
H, HD = 16, 128
NCORES = 8
HL = H // NCORES            # heads per core = 2
HDL = HL * HD               # local head dims = 256
T = B * S                   # 4096 tokens (b-major)
TCB = S // NCORES           # 256 tokens per core per batch after AllToAll
SCALE = float(1.0 / np.sqrt(HD))
BF = mybir.dt.bfloat16
F32 = mybir.dt.float32
AF = mybir.ActivationFunctionType


def build():
    nc = bacc.Bacc(None)
    xT = nc.declare_dram_parameter("xT", [D, T], BF, isOutput=False)
    # weights host-pretiled: [128, kb * out_cols] with kb-major columns
    wq = nc.declare_dram_parameter("wq", [128, 16 * HDL], BF, isOutput=False)
    wk = nc.declare_dram_parameter("wk", [128, 16 * HDL], BF, isOutput=False)
    wv = nc.declare_dram_parameter("wv", [128, 16 * HDL], BF, isOutput=False)
    wo = nc.declare_dram_parameter("wo", [128, 16 * D], BF, isOutput=False)
    cosT = nc.declare_dram_parameter("cosT", [HD, S], BF, isOutput=False)
    sinmT = nc.declare_dram_parameter("sinmT", [HD, S], BF, isOutput=False)
    maskT = nc.declare_dram_parameter("maskT", [128, 128], BF, isOutput=False)
    out = nc.declare_dram_parameter("out", [2 * TCB, D], F32, isOutput=True)

    with tile.TileContext(nc) as tc:
        with tc.tile_pool(name="persist", bufs=1) as persist, \
             tc.tile_pool(name="dram", bufs=1, space="DRAM") as dram, \
             tc.tile_pool(name="psA", bufs=3, space="PSUM") as psA, \
             tc.tile_pool(name="psO", bufs=2, space="PSUM") as psO, \
             tc.tile_pool(name="psD", bufs=2, space="PSUM") as psD, \
             tc.tile_pool(name="psP", bufs=1, space="PSUM") as psP, \
             tc.tile_pool(name="expp", bufs=8) as expp, \
             tc.tile_pool(name="yp", bufs=3) as yp, \
             tc.tile_pool(name="recp", bufs=2) as recp:

            cos_sb = persist.tile([HD, S], BF, tag="cos")
            sinm_sb = persist.tile([HD, S], BF, tag="sinm")
            mask_sb = persist.tile([128, 128], BF, tag="mask")
            ones_sb = persist.tile([128, 128], BF, tag="ones")
            qT_sb = [persist.tile([128, T], BF, name=f"qT{h}", tag=f"qT{h}") for h in range(HL)]
            kT_sb = [persist.tile([128, T], BF, name=f"kT{h}", tag=f"kT{h}") for h in range(HL)]
            v_sb = [persist.tile([128, HDL], BF, name=f"v{i}", tag=f"v{i}") for i in range(T // 128)]
            # per-(batch, token-half) AllToAll bounce buffers
            a2a_in = [[dram.tile([NCORES, HDL, 128], BF, name=f"a2a_in{b}_{hf}", tag=f"a2a_in{b}_{hf}")
                       for hf in range(2)] for b in range(B)]
            a2a_out = [[dram.tile([NCORES, HDL, 128], BF, name=f"a2a_out{b}_{hf}", tag=f"a2a_out{b}_{hf}")
                        for hf in range(2)] for b in range(B)]

            # warmup collective: absorbs the ~30us first-collective ncfw
            # setup cost while phase 1 computes
            dum_in = dram.tile([NCORES, 1, 128], BF, tag="dum_in")
            dum_out = dram.tile([NCORES, 1, 128], BF, tag="dum_out")
            nc.gpsimd.collective_compute(
                "AllToAll", mybir.AluOpType.bypass,
                ins=[dum_in.opt()], outs=[dum_out.opt()],
                replica_groups=[list(range(NCORES))])

            # ----- phase 1: QKV projections + RoPE -----
            with tc.tile_pool(name="xp", bufs=36) as xp, \
                 tc.tile_pool(name="wp", bufs=1) as wp, \
                 tc.tile_pool(name="tmp", bufs=4) as tmpp:
                # x arrives as [kb, n-chunk] tiles of [128, 512] so the first
                # projection PSUM only waits on 2 MB, not the whole half
                xt = {}
                for th in range(2):
                    for n in range(4):
                        xt[th, n] = [xp.tile([128, 512], BF, name=f"xt{th}_{n}_{i}", tag="x")
                                     for i in range(16)]
                w_sb = {}
                wdma_emitted = False

                def emit_wdmas():
                    for name, wparam in (("q", wq), ("k", wk), ("v", wv)):
                        wt = wp.tile([128, 16 * HDL], BF, name=f"w{name}", tag=f"w{name}")
                        for qd in range(4):
                            nc.gpsimd.dma_start(
                                wt[:, qd * 1024:(qd + 1) * 1024],
                                wparam[:, qd * 1024:(qd + 1) * 1024])
                        w_sb[name] = wt
                    nc.gpsimd.dma_start(cos_sb[:], cosT[:, :])
                    nc.gpsimd.dma_start(sinm_sb[:], sinmT[:, :])
                    nc.gpsimd.dma_start(mask_sb[:], maskT[:, :])
                    nc.vector.memset(ones_sb[:], 1.0)

                for th in range(2):
                    for n in range(4):
                        for kb in range(16):
                            eng = nc.sync if kb % 2 == 0 else nc.scalar
                            eng.dma_start(
                                xt[th, n][kb][:],
                                xT[kb * 128:(kb + 1) * 128,
                                   th * 2048 + n * 512: th * 2048 + (n + 1) * 512])
                        if not wdma_emitted:
                            wdma_emitted = True
                            emit_wdmas()

                for th in range(2):  # token half == batch index
                    for n in range(4):
                        # Q, K in d-major layout with fused RoPE on eviction
                        for name, dest in (("q", qT_sb), ("k", kT_sb)):
                            for m in range(HL):
                                ps = psA.tile([128, 512], F32, tag="a")
                                for kb in range(16):
                                    nc.tensor.matmul(
                                        ps[:],
                                        w_sb[name][:, kb * HDL + m * 128: kb * HDL + (m + 1) * 128],
                                        xt[th, n][kb][:],
                                        start=(kb == 0), stop=(kb == 15))
                                t1 = tmpp.tile([128, 512], F32, tag="t1")
                                t2 = tmpp.tile([128, 512], F32, tag="t2")
                                tg = slice(n * 512, (n + 1) * 512)
                                nc.vector.tensor_mul(t1[:], ps[:], cos_sb[:, tg])
                                nc.vector.tensor_mul(t2[0:64, :], ps[64:128, :], sinm_sb[0:64, tg])
                                nc.vector.tensor_mul(t2[64:128, :], ps[0:64, :], sinm_sb[64:128, tg])
                                dst = dest[m][:, th * 2048 + n * 512: th * 2048 + (n + 1) * 512]
                                nc.vector.tensor_add(dst, t1[:], t2[:])
                        # V in token-major layout
                        for tmi in range(4):
                            tm = n * 4 + tmi
                            psv = psA.tile([128, 512], F32, tag="a")
                            for kb in range(16):
                                nc.tensor.matmul(
                                    psv[:, 0:HDL],
                                    xt[th, n][kb][:, tmi * 128:(tmi + 1) * 128],
                                    w_sb["v"][:, kb * HDL:(kb + 1) * HDL],
                                    start=(kb == 0), stop=(kb == 15))
                            nc.scalar.copy(v_sb[th * 16 + tm][:], psv[:, 0:HDL])

            with tc.tile_pool(name="wop", bufs=1) as wop, \
                 tc.tile_pool(name="ytp", bufs=1) as ytp, \
                 tc.tile_pool(name="outp", bufs=3) as outp:
                # wo lands in the space freed by phase 1 and overlaps attention
                wo_sb = wop.tile([128, 16 * D], BF, tag="wo")
                for half in range(2):
                    nc.scalar.dma_start(
                        wo_sb[:, half * 8 * D:(half + 1) * 8 * D],
                        wo[:, half * 8 * D:(half + 1) * 8 * D])

                def out_proj(b, hf):
                    """project my 128-token chunk of (batch b, half hf)"""
                    yt = ytp.tile([128, 16 * 128], BF, name=f"yt{b}_{hf}", tag=f"yt{b}_{hf}")
                    nc.gpsimd.dma_start(
                        yt[:],
                        a2a_out[b][hf].rearrange("r (h d) t -> d r h t", h=2))
                    for ncol in range(4):
                        ps = psP.tile([128, 512], F32, tag="p")
                        for kb in range(16):
                            mm = nc.tensor.matmul(
                                ps[:],
                                yt[:, kb * 128:(kb + 1) * 128],
                                wo_sb[:, kb * D + ncol * 512: kb * D + (ncol + 1) * 512],
                                start=(kb == 0), stop=(kb == 15))
                            if kb == 0 and "mm" in last_attn:
                                tile.add_dep_helper(
                                    getattr(mm, "ins", mm),
                                    getattr(last_attn["mm"], "ins", last_attn["mm"]),
                                    sync=False,
                                    reason="out_proj after attention: PE saturated")
                        o = outp.tile([128, 512], F32, tag="ot")
                        nc.scalar.copy(o[:], ps[:])
                        nc.gpsimd.dma_start(
                            out[b * TCB + hf * 128: b * TCB + hf * 128 + 128,
                                ncol * 512:(ncol + 1) * 512],
                            o[:])

                last_attn = {}
                # ----- phase 2: causal attention in [k, q] layout -----
                for b in range(B):
                    for qsb in range(4):
                        for h in range(HL):
                            out_ps = psO.tile([128, 512], F32, tag="o")
                            den_ps = psD.tile([128, 512], F32, tag="d")
                            nkb = (qsb + 1) * 4
                            for kb in range(nkb):
                                j = kb - qsb * 4  # >=0 on the causal diagonal band
                                c0 = max(j, 0) * 128  # first valid q column
                                lg = psA.tile([128, 512], F32, tag="a")
                                nc.tensor.matmul(
                                    lg[:, c0:512],
                                    kT_sb[h][:, b * S + kb * 128: b * S + (kb + 1) * 128],
                                    qT_sb[h][:, b * S + qsb * 512 + c0: b * S + (qsb + 1) * 512],
                                    start=True, stop=True)
                                pe = expp.tile([128, 512], BF, tag="e")
                                nc.scalar.activation(pe[:, c0:512], lg[:, c0:512],
                                                     AF.Exp, scale=SCALE)
                                if j >= 0:
                                    nc.vector.tensor_mul(
                                        pe[:, c0:c0 + 128], pe[:, c0:c0 + 128], mask_sb[:])
                                nc.tensor.matmul(
                                    out_ps[:, c0:512],
                                    v_sb[b * 16 + kb][:, h * 128:(h + 1) * 128],
                                    pe[:, c0:512],
                                    start=(kb == 0), stop=(kb == nkb - 1))
                                last_attn["mm"] = nc.tensor.matmul(
                                    den_ps[:, c0:512], ones_sb[:], pe[:, c0:512],
                                    start=(kb == 0), stop=(kb == nkb - 1))
                            rec = recp.tile([128, 512], F32, tag="r")
                            nc.vector.reciprocal_approx_fast(rec[:], den_ps[:])
                            y = yp.tile([128, 512], BF, tag="y")
                            nc.vector.tensor_mul(y[:], out_ps[:], rec[:])
                            for jj in range(4):
                                nc.sync.dma_start(
                                    a2a_in[b][qsb // 2][(qsb % 2) * 4 + jj,
                                                        h * 128:(h + 1) * 128, :],
                                    y[:, jj * 128:(jj + 1) * 128])
                        if qsb % 2 == 1:
                            nc.gpsimd.collective_compute(
                                "AllToAll", mybir.AluOpType.bypass,
                                ins=[a2a_in[b][qsb // 2].opt()],
                                outs=[a2a_out[b][qsb // 2].opt()],
                                replica_groups=[list(range(NCORES))])
                # emitted last so they gap-fill the attention phase once
                # their AllToAll lands
                out_proj(0, 0)
                out_proj(0, 1)
                out_proj(1, 0)
                out_proj(1, 1)
    nc.finalize()
    return nc


_CACHE = {}


def _get_nc():
    if "nc" not in _CACHE:
        _CACHE["nc"] = build()
    return _CACHE["nc"]


def _pretile(w, cols):
    """[2048, cols] -> [128, 16*cols], kb-major along columns."""
    return np.ascontiguousarray(
        w.reshape(16, 128, cols).transpose(1, 0, 2).reshape(128, 16 * cols)
    ).astype(BF16)


def _prep_in_maps(x, freq_cos, freq_sin, wq, wk, wv, wo):
    xTa = np.asarray(x, np.float32).reshape(T, D).T.astype(BF16)
    cos_t = np.asarray(freq_cos, np.float32).T  # [64, S]
    sin_t = np.asarray(freq_sin, np.float32).T
    cosT = np.concatenate([cos_t, cos_t], 0).astype(BF16)
    sinmT = np.concatenate([-sin_t, sin_t], 0).astype(BF16)
    kk = np.arange(128)[:, None]
    qq = np.arange(128)[None, :]
    maskTa = (qq >= kk).astype(BF16)
    wo_t = _pretile(np.asarray(wo, np.float32), D)
    wq = np.asarray(wq, np.float32)
    wk = np.asarray(wk, np.float32)
    wv = np.asarray(wv, np.float32)
    in_maps = []
    for c in range(NCORES):
        sl = slice(c * HDL, (c + 1) * HDL)
        in_maps.append({
            "xT": xTa,
            "wq": _pretile(wq[:, sl], HDL),
            "wk": _pretile(wk[:, sl], HDL),
            "wv": _pretile(wv[:, sl], HDL),
            "wo": wo_t,
            "cosT": cosT,
            "sinmT": sinmT,
            "maskT": maskTa,
        })
    return in_maps


def kernel(x, freq_cos, freq_sin, wq, wk, wv, wo, _trace=False):
    nc = _get_nc()
    in_maps = _prep_in_maps(x, freq_cos, freq_sin, wq, wk, wv, wo)
    res = run_bass_kernel_spmd(
        nc, in_maps, core_ids=list(range(NCORES)), trace=_trace)
    # core c holds tokens [hf*1024 + c*128) of each (batch, half)
    full = np.empty((B, S, D), np.float32)
    for c in range(NCORES):
        o = np.asarray(res.results[c]["out"], np.float32)
        for b in range(B):
            for hf in range(2):
                full[b, hf * 1024 + c * 128: hf * 1024 + (c + 1) * 128] = \
                    o[b * TCB + hf * 128: b * TCB + hf * 128 + 128]
    if _trace:
        return full, res
    return full


# revision 14
# speedup vs baseline: 1.1788x; 1.0271x over previous
"""Dense causal attention block (QKV proj + RoPE + causal attention + out proj)
distributed over 8 TRN2 NeuronCores.

Sharding: tensor-parallel over heads (2 heads/core). Each core computes
q/k/v for its heads (q/k d-major so QK^T needs no transposes, v
token-major), causal attention in logits-transposed [k, q] layout
(softmax denominator via an all-ones stationary matmul, no max
subtraction -- logits are O(1) -- normalization folded into the PSUM
eviction), then one AllToAll per batch converts head-sharding to
token-sharding so the output projection overlaps the other batch's
attention. Host assembles the 16 token chunks.

All matmuls run in bf16 (f32 PSUM accumulate); host pre-casts and
pre-tiles the weights so every DMA is partition-contiguous.
"""
import numpy as np
import ml_dtypes

import concourse.bass as bass
import concourse.mybir as mybir
from concourse import tile, bacc
from concourse.bass_utils import run_bass_kernel_spmd

BF16 = ml_dtypes.bfloat16
B, S, D = 2, 2048, 2048
H, HD = 16, 128
NCORES = 8
HL = H // NCORES            # heads per core = 2
HDL = HL * HD               # local head dims = 256
T = B * S                   # 4096 tokens (b-major)
TCB = S // NCORES           # 256 tokens per core per batch after AllToAll
SCALE = float(1.0 / np.sqrt(HD))
BF = mybir.dt.bfloat16
F32 = mybir.dt.float32
AF = mybir.ActivationFunctionType


def build():
    nc = bacc.Bacc(None)
    xT = nc.declare_dram_parameter("xT", [D, T], BF, isOutput=False)
    # weights host-pretiled: [128, kb * out_cols] with kb-major columns
    wq = nc.declare_dram_parameter("wq", [128, 16 * HDL], BF, isOutput=False)
    wk = nc.declare_dram_parameter("wk", [128, 16 * HDL], BF, isOutput=False)
    wv = nc.declare_dram_parameter("wv", [128, 16 * HDL], BF, isOutput=False)
    wo = nc.declare_dram_parameter("wo", [128, 16 * D], BF, isOutput=False)
    cosT = nc.declare_dram_parameter("cosT", [HD, S], BF, isOutput=False)
    sinmT = nc.declare_dram_parameter("sinmT", [HD, S], BF, isOutput=False)
    maskT = nc.declare_dram_parameter("maskT", [128, 128], BF, isOutput=False)
    out = nc.declare_dram_parameter("out", [2 * TCB, D], F32, isOutput=True)

    with tile.TileContext(nc) as tc:
        with tc.tile_pool(name="persist", bufs=1) as persist, \
             tc.tile_pool(name="dram", bufs=1, space="DRAM") as dram, \
             tc.tile_pool(name="psA", bufs=3, space="PSUM") as psA, \
             tc.tile_pool(name="psO", bufs=2, space="PSUM") as psO, \
             tc.tile_pool(name="psD", bufs=2, space="PSUM") as psD, \
             tc.tile_pool(name="psP", bufs=1, space="PSUM") as psP, \
             tc.tile_pool(name="expp", bufs=8) as expp, \
             tc.tile_pool(name="yp", bufs=3) as yp, \
             tc.tile_pool(name="recp", bufs=2) as recp:

            cos_sb = persist.tile([HD, S], BF, tag="cos")
            sinm_sb = persist.tile([HD, S], BF, tag="sinm")
            mask_sb = persist.tile([128, 128], BF, tag="mask")
            ones_sb = persist.tile([128, 128], BF, tag="ones")
            qT_sb = [persist.tile([128, T], BF, name=f"qT{h}", tag=f"qT{h}") for h in range(HL)]
            kT_sb = [persist.tile([128, T], BF, name=f"kT{h}", tag=f"kT{h}") for h in range(HL)]
            v_sb = [persist.tile([128, HDL], BF, name=f"v{i}", tag=f"v{i}") for i in range(T // 128)]
            # per-(batch, token-half) AllToAll bounce buffers
            a2a_in = [[dram.tile([NCORES, HDL, 128], BF, name=f"a2a_in{b}_{hf}", tag=f"a2a_in{b}_{hf}")
                       for hf in range(2)] for b in range(B)]
            a2a_out = [[dram.tile([NCORES, HDL, 128], BF, name=f"a2a_out{b}_{hf}", tag=f"a2a_out{b}_{hf}")
                        for hf in range(2)] for b in range(B)]

            # warmup collective: absorbs the ~30us first-collective ncfw
            # setup cost while phase 1 computes
            dum_in = dram.tile([NCORES, 1, 128], BF, tag="dum_in")
            dum_out = dram.tile([NCORES, 1, 128], BF, tag="dum_out")
            nc.gpsimd.collective_compute(
                "AllToAll", mybir.AluOpType.bypass,
                ins=[dum_in.opt()], outs=[dum_out.opt()],
                replica_groups=[list(range(NCORES))])

            # ----- phase 1: QKV projections + RoPE -----
            with tc.tile_pool(name="xp", bufs=36) as xp, \
                 tc.tile_pool(name="wp", bufs=1) as wp, \
                 tc.tile_pool(name="tmp", bufs=4) as tmpp:
                # x arrives as [kb, n-chunk] tiles of [128, 512] so the first
                # projection PSUM only waits on 2 MB, not the whole half
                xt = {}
                for th in range(2):
                    for n in range(4):
                        xt[th, n] = [xp.tile([128, 512], BF, name=f"xt{th}_{n}_{i}", tag="x")
                                     for i in range(16)]
                w_sb = {}
                wdma_emitted = False

                def emit_wdmas():
                    for name, wparam in (("q", wq), ("k", wk), ("v", wv)):
                        wt = wp.tile([128, 16 * HDL], BF, name=f"w{name}", tag=f"w{name}")
                        for qd in range(4):
                            nc.gpsimd.dma_start(
                                wt[:, qd * 1024:(qd + 1) * 1024],
                                wparam[:, qd * 1024:(qd + 1) * 1024])
                        w_sb[name] = wt
                    nc.gpsimd.dma_start(cos_sb[:], cosT[:, :])
                    nc.gpsimd.dma_start(sinm_sb[:], sinmT[:, :])
                    nc.gpsimd.dma_start(mask_sb[:], maskT[:, :])
                    nc.vector.memset(ones_sb[:], 1.0)

                for th in range(2):
                    for n in range(4):
                        for kb in range(16):
                            eng = nc.sync if kb % 2 == 0 else nc.scalar
                            eng.dma_start(
                                xt[th, n][kb][:],
                                xT[kb * 128:(kb + 1) * 128,
                                   th * 2048 + n * 512: th * 2048 + (n + 1) * 512])
                        if not wdma_emitted:
                            wdma_emitted = True
                            emit_wdmas()

                for th in range(2):  # token half == batch index
                    for n in range(4):
                        # Q, K in d-major layout with fused RoPE on eviction
                        for name, dest in (("q", qT_sb), ("k", kT_sb)):
                            for m in range(HL):
                                ps = psA.tile([128, 512], F32, tag="a")
                                for kb in range(16):
                                    nc.tensor.matmul(
                                        ps[:],
                                        w_sb[name][:, kb * HDL + m * 128: kb * HDL + (m + 1) * 128],
                                        xt[th, n][kb][:],
                                        start=(kb == 0), stop=(kb == 15))
                                t1 = tmpp.tile([128, 512], F32, tag="t1")
                                t2 = tmpp.tile([128, 512], F32, tag="t2")
                                tg = slice(n * 512, (n + 1) * 512)
                                nc.vector.tensor_mul(t1[:], ps[:], cos_sb[:, tg])
                                nc.vector.tensor_mul(t2[0:64, :], ps[64:128, :], sinm_sb[0:64, tg])
                                nc.vector.tensor_mul(t2[64:128, :], ps[0:64, :], sinm_sb[64:128, tg])
                                dst = dest[m][:, th * 2048 + n * 512: th * 2048 + (n + 1) * 512]
                                nc.vector.tensor_add(dst, t1[:], t2[:])
                        # V in token-major layout
                        for tmi in range(4):
                            tm = n * 4 + tmi
                            psv = psA.tile([128, 512], F32, tag="a")
                            for kb in range(16):
                                nc.tensor.matmul(
                                    psv[:, 0:HDL],
                                    xt[th, n][kb][:, tmi * 128:(tmi + 1) * 128],
                                    w_sb["v"][:, kb * HDL:(kb + 1) * HDL],
                                    start=(kb == 0), stop=(kb == 15))
                            nc.scalar.copy(v_sb[th * 16 + tm][:], psv[:, 0:HDL])

            with tc.tile_pool(name="wop", bufs=1) as wop, \
                 tc.tile_pool(name="ytp", bufs=1) as ytp, \
                 tc.tile_pool(name="outp", bufs=3) as outp:
                # wo lands in the space freed by phase 1 and overlaps attention
                wo_sb = wop.tile([128, 16 * D], BF, tag="wo")
                for half in range(2):
                    nc.scalar.dma_start(
                        wo_sb[:, half * 8 * D:(half + 1) * 8 * D],
                        wo[:, half * 8 * D:(half + 1) * 8 * D])

                yt_sb = {}

                def out_proj(b, hf):
                    """project my 128-token chunk of (batch b, half hf)"""
                    yt = yt_sb[b, hf]
                    for ncol in range(4):
                        ps = psP.tile([128, 512], F32, tag="p")
                        for kb in range(16):
                            mm = nc.tensor.matmul(
                                ps[:],
                                yt[:, kb * 128:(kb + 1) * 128],
                                wo_sb[:, kb * D + ncol * 512: kb * D + (ncol + 1) * 512],
                                start=(kb == 0), stop=(kb == 15))
                            if kb == 0 and "mm" in last_attn:
                                tile.add_dep_helper(
                                    getattr(mm, "ins", mm),
                                    getattr(last_attn["mm"], "ins", last_attn["mm"]),
                                    sync=False,
                                    reason="out_proj after attention: PE saturated")
                        o = outp.tile([128, 512], F32, tag="ot")
                        nc.scalar.copy(o[:], ps[:])
                        nc.sync.dma_start(
                            out[b * TCB + hf * 128: b * TCB + hf * 128 + 128,
                                ncol * 512:(ncol + 1) * 512],
                            o[:])

                last_attn = {}
                # ----- phase 2: causal attention in [k, q] layout -----
                for b in range(B):
                    for qsb in range(4):
                        for h in range(HL):
                            out_ps = psO.tile([128, 512], F32, tag="o")
                            den_ps = psD.tile([128, 512], F32, tag="d")
                            nkb = (qsb + 1) * 4
                            for kb in range(nkb):
                                j = kb - qsb * 4  # >=0 on the causal diagonal band
                                c0 = max(j, 0) * 128  # first valid q column
                                lg = psA.tile([128, 512], F32, tag="a")
                                nc.tensor.matmul(
                                    lg[:, c0:512],
                                    kT_sb[h][:, b * S + kb * 128: b * S + (kb + 1) * 128],
                                    qT_sb[h][:, b * S + qsb * 512 + c0: b * S + (qsb + 1) * 512],
                                    start=True, stop=True)
                                pe = expp.tile([128, 512], BF, tag="e")
                                nc.scalar.activation(pe[:, c0:512], lg[:, c0:512],
                                                     AF.Exp, scale=SCALE)
                                if j >= 0:
                                    nc.vector.tensor_mul(
                                        pe[:, c0:c0 + 128], pe[:, c0:c0 + 128], mask_sb[:])
                                nc.tensor.matmul(
                                    out_ps[:, c0:512],
                                    v_sb[b * 16 + kb][:, h * 128:(h + 1) * 128],
                                    pe[:, c0:512],
                                    start=(kb == 0), stop=(kb == nkb - 1))
                                last_attn["mm"] = nc.tensor.matmul(
                                    den_ps[:, c0:512], ones_sb[:], pe[:, c0:512],
                                    start=(kb == 0), stop=(kb == nkb - 1))
                            rec = recp.tile([128, 512], F32, tag="r")
                            nc.vector.reciprocal_approx_fast(rec[:], den_ps[:])
                            y = yp.tile([128, 512], BF, tag="y")
                            nc.vector.tensor_mul(y[:], out_ps[:], rec[:])
                            for jj in range(4):
                                nc.sync.dma_start(
                                    a2a_in[b][qsb // 2][(qsb % 2) * 4 + jj,
                                                        h * 128:(h + 1) * 128, :],
                                    y[:, jj * 128:(jj + 1) * 128])
                        if qsb % 2 == 1:
                            hf = qsb // 2
                            nc.gpsimd.collective_compute(
                                "AllToAll", mybir.AluOpType.bypass,
                                ins=[a2a_in[b][hf].opt()],
                                outs=[a2a_out[b][hf].opt()],
                                replica_groups=[list(range(NCORES))])
                            yt = ytp.tile([128, 16 * 128], BF,
                                          name=f"yt{b}_{hf}", tag=f"yt{b}_{hf}")
                            nc.gpsimd.dma_start(
                                yt[:],
                                a2a_out[b][hf].rearrange("r (h d) t -> d r h t", h=2))
                            yt_sb[b, hf] = yt
                # emitted last so they gap-fill the attention phase once
                # their AllToAll lands
                out_proj(0, 0)
                out_proj(0, 1)
                out_proj(1, 0)
                out_proj(1, 1)
    nc.finalize()
    return nc


_CACHE = {}


def _get_nc():
    if "nc" not in _CACHE:
        _CACHE["nc"] = build()
    return _CACHE["nc"]


def _pretile(w, cols):
    """[2048, cols] -> [128, 16*cols], kb-major along columns."""
    return np.ascontiguousarray(
        w.reshape(16, 128, cols).transpose(1, 0, 2).reshape(128, 16 * cols)
    ).astype(BF16)


def _prep_in_maps(x, freq_cos, freq_sin, wq, wk, wv, wo):
    xTa = np.asarray(x, np.float32).reshape(T, D).T.astype(BF16)
    cos_t = np.asarray(freq_cos, np.float32).T  # [64, S]
    sin_t = np.asarray(freq_sin, np.float32).T
    cosT = np.concatenate([cos_t, cos_t], 0).astype(BF16)
    sinmT = np.concatenate([-sin_t, sin_t], 0).astype(BF16)
    kk = np.arange(128)[:, None]
    qq = np.arange(128)[None, :]
    maskTa = (qq >= kk).astype(BF16)
    wo_t = _pretile(np.asarray(wo, np.float32), D)
    wq = np.asarray(wq, np.float32)
    wk = np.asarray(wk, np.float32)
    wv = np.asarray(wv, np.float32)
    in_maps = []
    for c in range(NCORES):
        sl = slice(c * HDL, (c + 1) * HDL)
        in_maps.append({
            "xT": xTa,
            "wq": _pretile(wq[:, sl], HDL),
            "wk": _pretile(wk[:, sl], HDL),
            "wv": _pretile(wv[:, sl], HDL),
            "wo": wo_t,
            "cosT": cosT,
            "sinmT": sinmT,
            "maskT": maskTa,
        })
    return in_maps


def kernel(x, freq_cos, freq_sin, wq, wk, wv, wo, _trace=False):
    nc = _get_nc()
    in_maps = _prep_in_maps(x, freq_cos, freq_sin, wq, wk, wv, wo)
    res = run_bass_kernel_spmd(
        nc, in_maps, core_ids=list(range(NCORES)), trace=_trace)
    # core c holds tokens [hf*1024 + c*128) of each (batch, half)
    full = np.empty((B, S, D), np.float32)
    for c in range(NCORES):
        o = np.asarray(res.results[c]["out"], np.float32)
        for b in range(B):
            for hf in range(2):
                full[b, hf * 1024 + c * 128: hf * 1024 + (c + 1) * 128] = \
                    o[b * TCB + hf * 128: b * TCB + hf * 128 + 128]
    if _trace:
        return full, res
    return full
